# revision 21
# baseline (speedup 1.0000x reference)
"""Cross-attention kernel for 8 trn2 NeuronCores.

Problem: B=2, Lq=Lk=2048, D=1024, H=16, dh=64.
  q/k/v = Linear(x); q,k L2-normalized per head; S = q@k.T * 1/8;
  key-pad mask -> -1e9; softmax; mask-aware renorm; eps-smooth toward
  uniform-over-valid; out = attn@v merged -> out_proj.

Sharding: core c handles batch b=c//4, heads [4*(c%4), 4*(c%4)+4)
(two "head pairs" hp of 2 heads each). Each core computes a partial
output-projection over its 256 head dims; the host sums the 8 partials
(4 per batch) and adds the output bias.

Key optimizations over the v1 kernel:
  - Key compaction: masked keys contribute exactly 0 (exp underflows to
    0), so the host gathers only the valid keys (~50%) and pads to a
    multiple of 128.  S / exp / AV / k,v projections all shrink
    proportionally.  The compiled graph is cached per padded-key-tile
    count KT_C.
  - Fused rowsum: the AV matmul's stationary matrix is [v_h | ones]
    column-blocks, so PSUM partitions 64:128 accumulate the softmax
    denominator replicated across 64 partitions -- no separate rowsum
    matmuls and no cross-partition broadcast for the divide.
  - Inputs staged with few large DMAs; out-projection interleaved into
    the attention loop; elementwise work split across DVE/ACT/GpSimd.

Math notes (equivalences used, all within fp rounding of the reference):
  - logits are bounded (|q̂·k̂|/8 <= 0.125) so softmax max-subtraction is
    skipped; masked/pad keys get an additive -30000 bias inside the exp
    (per-key bias = per-partition bias in the transposed S layout), which
    underflows exp to exactly 0 like the reference's -1e9 path.
  - softmax + mask-zero + renorm == (exp @ v) / rowsum(exp) since masked
    entries are exactly 0.
  - eps smoothing: attn' = 0.9*attn + 0.1*valid/nv, so
    out = 0.9*(P@v)/rs + 0.1*vmean; vmean*0.1 is computed on the host
    from v_in/Wv/bv exactly.
"""

import ml_dtypes
import numpy as np

import concourse.bass as bass
from concourse import bacc
import concourse.mybir as mybir
import concourse.tile as tile
from concourse.bass_utils import run_bass_kernel_spmd

F32 = mybir.dt.float32
BF16 = mybir.dt.bfloat16
AF = mybir.ActivationFunctionType
ALU = mybir.AluOpType

B, L, D = 2, 2048, 1024
H, DH = 16, 64
HEADS_PER_CORE = 4          # -> 256 dims per core, 2 head-pairs
HPC = HEADS_PER_CORE * DH   # 256
SCALE = 0.125               # 1/sqrt(64) / ATTN_TEMP
EPS_SMOOTH = 0.1
MASK_BIAS = -30000.0
N_CORES = 8
QC = L // 512               # 4 q chunks
NCH = D // 128              # 8 contraction chunks for projections


def _chunks(total, step=512):
    out, s = [], 0
    while s < total:
        cs = min(step, total - s)
        out.append((s, cs))
        s += cs
    return out


def _build_nc(kt_c):
    LKC = kt_c * 128
    nc = bacc.Bacc(None)

    xqT = nc.dram_tensor("xqT", [D, L], BF16, kind="ExternalInput")
    xkT = nc.dram_tensor("xkT", [D, LKC], BF16, kind="ExternalInput")
    xvT = nc.dram_tensor("xvT", [D, LKC], BF16, kind="ExternalInput")
    wq_t = nc.dram_tensor("wq_t", [D, HPC], BF16, kind="ExternalInput")
    wk_t = nc.dram_tensor("wk_t", [D, HPC], BF16, kind="ExternalInput")
    wv_t = nc.dram_tensor("wv_t", [D, HPC], BF16, kind="ExternalInput")
    wo_t = nc.dram_tensor("wo_t", [HPC, D], BF16, kind="ExternalInput")
    bq = nc.dram_tensor("bq", [2, 1, 128], BF16, kind="ExternalInput")
    bk = nc.dram_tensor("bk", [2, 1, 128], BF16, kind="ExternalInput")
    bv = nc.dram_tensor("bv", [1, HPC], BF16, kind="ExternalInput")
    mbias = nc.dram_tensor("mbias", [128, kt_c], F32, kind="ExternalInput")
    vmean = nc.dram_tensor("vmean", [2, 128, 1], F32, kind="ExternalInput")
    partial = nc.dram_tensor("partial", [L, D], F32, kind="ExternalOutput")

    with tile.TileContext(nc) as tc:
        with (
            tc.tile_pool(name="consts", bufs=1) as consts,
            tc.tile_pool(name="wpool", bufs=1) as wpool,
            tc.tile_pool(name="xpool", bufs=1) as xpool,
            tc.tile_pool(name="persist", bufs=1) as persist,
            tc.tile_pool(name="l2pool", bufs=4) as l2pool,
            tc.tile_pool(name="ppool", bufs=4) as ppool,
            tc.tile_pool(name="divpool", bufs=4) as divpool,
            tc.tile_pool(name="ostpool", bufs=4) as ostpool,
            tc.tile_pool(name="ps", bufs=2, space="PSUM") as ps,
        ):
            # ---- weights / consts / inputs, DMA-issued in consumption
            # order: the whole k-projection group first, then v, then q ----
            w_sb = {}
            bias_sb = {}
            xk_sb = xpool.tile([128, NCH, LKC], BF16, tag="xk")
            xv_sb = xpool.tile([128, NCH, LKC], BF16, tag="xv")
            xq_sb = xpool.tile([128, NCH, L], BF16, tag="xq")

            def load_group(name, whnd, bhnd, x_sb, xhnd):
                # split transfers across several DMAs so multiple DMA queues
                # move the data in parallel
                xr = xhnd.rearrange("(c p) t -> p c t", p=128)
                for c in range(0, NCH, 2):
                    nc.sync.dma_start(out=x_sb[:, c:c + 2, :],
                                      in_=xr[:, c:c + 2, :])
                t = wpool.tile([128, NCH, HPC], BF16, tag=f"w{name}",
                               name=f"w{name}")
                wr = whnd.rearrange("(c p) m -> p c m", p=128)
                for c in range(0, NCH, 4):
                    nc.sync.dma_start(out=t[:, c:c + 4, :],
                                      in_=wr[:, c:c + 4, :])
                w_sb[name] = t
                if bhnd is not None:
                    for hp in range(2):
                        bt = consts.tile([1, 128], BF16, tag=f"b{name}{hp}",
                                         name=f"b{name}{hp}")
                        nc.sync.dma_start(out=bt, in_=bhnd[hp])
                        bias_sb[(name, hp)] = bt

            load_group("k", wk_t, bk, xk_sb, xkT)
            load_group("v", wv_t, None, xv_sb, xvT)
            bv_sb = consts.tile([1, HPC], BF16, tag="bv")
            nc.sync.dma_start(out=bv_sb, in_=bv[:, :])
            mbias_sb = consts.tile([128, kt_c], F32, tag="mbias")
            nc.sync.dma_start(out=mbias_sb, in_=mbias[:, :])
            vmean_sb = []
            for hp in range(2):
                t = consts.tile([128, 1], F32, tag=f"vmean{hp}",
                                name=f"vmean{hp}")
                nc.sync.dma_start(out=t, in_=vmean[hp])
                vmean_sb.append(t)
            load_group("q", wq_t, bq, xq_sb, xqT)
            wo_sb = wpool.tile([128, 2, D], BF16, tag="wo")
            nc.sync.dma_start(
                out=wo_sb, in_=wo_t.rearrange("(h p) m -> p h m", p=128)
            )

            ones_row = consts.tile([1, 512], BF16, tag="ones_row")
            nc.vector.memset(ones_row, 1.0)
            blockdiag = consts.tile([128, 128], BF16, tag="blockdiag")
            nc.vector.memset(blockdiag, 0.0)
            nc.vector.memset(blockdiag[0:64, 0:64], 1.0)
            nc.vector.memset(blockdiag[64:128, 64:128], 1.0)

            # ---- persistent activations ----
            qTn = [persist.tile([128, L], BF16, tag=f"qTn{hp}", name=f"qTn{hp}")
                   for hp in range(2)]
            kTn = [persist.tile([128, LKC], BF16, tag=f"kTn{hp}",
                                name=f"kTn{hp}")
                   for hp in range(2)]
            # [keys, kt, head, 0:64 ones | 64:128 v] -> fused rowsum + AV
            # (rowsum lands on PSUM partitions 0:64 where the reciprocal can
            # read it directly; O lands on 64:128 whose PSUM base is
            # independent of the SBUF operand base in the divide)
            v_aug = persist.tile([128, kt_c, HEADS_PER_CORE, 128], BF16,
                                 tag="v_aug")
            nc.vector.memset(v_aug[:, :, :, 0:64], 1.0)
            ofin = [persist.tile([128, L], BF16, tag=f"ofin{hp}",
                                 name=f"ofin{hp}")
                    for hp in range(2)]

            def proj_block(name, x_sb, dst, ts, cs):
                """Project+L2-normalize one token chunk of q or k."""
                tsl = slice(ts, ts + cs)
                psums = [ps.tile([128, 512], F32, tag="o", bufs=3,
                                 name=f"pj{name}{hp}")
                         for hp in range(2)]
                for c in range(NCH):
                    for hp in range(2):
                        nc.tensor.matmul(
                            psums[hp][:, 0:cs],
                            lhsT=w_sb[name][:, c, hp * 128:(hp + 1) * 128],
                            rhs=x_sb[:, c, tsl],
                            start=(c == 0),
                            stop=False,
                        )
                for hp in range(2):
                    # + bias (broadcast along tokens via K=1 matmul)
                    nc.tensor.matmul(
                        psums[hp][:, 0:cs],
                        lhsT=bias_sb[(name, hp)],
                        rhs=ones_row[:, 0:cs],
                        start=False,
                        stop=True,
                    )
                    # L2 norm over each head's 64 dims; square and the final
                    # multiply run on DVE in bf16 (2x mode) to keep the ACT
                    # engine free for the attention exp stream
                    qraw = l2pool.tile([128, 512], BF16, tag="qraw")
                    nc.vector.tensor_copy(qraw[:, 0:cs], psums[hp][:, 0:cs])
                    sq = l2pool.tile([128, 512], BF16, tag="sq")
                    nc.vector.tensor_mul(sq[:, 0:cs], qraw[:, 0:cs],
                                         qraw[:, 0:cs])
                    n2 = ps.tile([128, 512], F32, tag="s", bufs=2, name="n2")
                    nc.tensor.matmul(
                        n2[:, 0:cs], lhsT=blockdiag, rhs=sq[:, 0:cs],
                        start=True, stop=True,
                    )
                    # rnorm = 1/sqrt(n2) = exp(-0.5*ln(n2)); Ln and Exp share
                    # one act table set, so no ACT_TABLE_LOAD thrash with the
                    # attention exp
                    nlog = l2pool.tile([128, 512], F32, tag="nlog")
                    nc.scalar.activation(nlog[:, 0:cs], n2[:, 0:cs], AF.Ln)
                    rnorm = l2pool.tile([128, 512], BF16, tag="rnorm")
                    nc.scalar.activation(rnorm[:, 0:cs], nlog[:, 0:cs],
                                         AF.Exp, scale=-0.5)
                    nc.vector.tensor_mul(
                        dst[hp][:, tsl], qraw[:, 0:cs], rnorm[:, 0:cs],
                    )

            # ---- k projection ----
            for ts, cs in _chunks(LKC):
                proj_block("k", xk_sb, kTn, ts, cs)

            # ---- v projection (into v_aug with ones columns) ----
            for tt in range(kt_c):
                vp = ps.tile([128, HPC], F32, tag="o", bufs=3, name="vp")
                for c in range(NCH):
                    nc.tensor.matmul(
                        vp,
                        lhsT=xv_sb[:, c, tt * 128:(tt + 1) * 128],
                        rhs=w_sb["v"][:, c, :],
                        start=(c == 0), stop=False,
                    )
                nc.tensor.matmul(
                    vp, lhsT=ones_row[:, 0:128], rhs=bv_sb,
                    start=False, stop=True,
                )
                nc.vector.tensor_copy(
                    v_aug[:, tt, :, 64:128],
                    vp.rearrange("p (h d) -> p h d", h=HEADS_PER_CORE),
                )

            # ---- q projection (first chunk up front, rest interleaved) ----
            proj_block("q", xq_sb, qTn, 0, 512)

            # ---- attention + interleaved out-projection ----
            # out-projection pieces are queued per q-chunk and emitted one at
            # a time inside the NEXT chunk's kt loop, so the in-order tensor
            # queue never stalls on the psum->sbuf copy of the previous piece
            pending = []

            def queue_out_proj(qc):
                for tt4 in range(4):
                    tsl = slice(qc * 512 + tt4 * 128,
                                qc * 512 + tt4 * 128 + 128)
                    for nh in range(2):
                        pending.append((tsl, slice(nh * 512, (nh + 1) * 512)))

            def emit_piece(tag="op", bufs=1):
                tsl, nsl = pending.pop(0)
                op = ps.tile([128, 512], F32, tag=tag, bufs=bufs, name="op")
                nc.tensor.matmul(
                    op, lhsT=ofin[0][:, tsl],
                    rhs=wo_sb[:, 0, nsl], start=True, stop=False,
                )
                nc.tensor.matmul(
                    op, lhsT=ofin[1][:, tsl],
                    rhs=wo_sb[:, 1, nsl], start=False, stop=True,
                )
                ost = ostpool.tile([128, 512], F32, tag="ost")
                nc.vector.tensor_copy(ost, op)
                nc.sync.dma_start(out=partial[tsl, nsl], in_=ost)

            for qc in range(QC):
                qsl = slice(qc * 512, (qc + 1) * 512)
                for hp in range(2):
                    o_ps = [ps.tile([128, 512], F32, tag="o", bufs=3,
                                    name=f"o{i}")
                            for i in range(2)]
                    for kt in range(kt_c):
                        ksl = slice(kt * 128, (kt + 1) * 128)
                        s_ps = ps.tile([128, 1024], F32, tag="s", bufs=2,
                                       name="s")
                        # S_T = k̂.T q̂ per head, row-packed (K=64 each)
                        nc.tensor.matmul(
                            s_ps[:, 0:512],
                            lhsT=kTn[hp][0:64, ksl],
                            rhs=qTn[hp][0:64, qsl],
                            start=True, stop=True,
                        )
                        nc.tensor.matmul(
                            s_ps[:, 512:1024],
                            lhsT=kTn[hp][64:128, ksl],
                            rhs=qTn[hp][64:128, qsl],
                            start=True, stop=True,
                        )
                        # P = exp(SCALE*S + pad_bias); masked keys -> 0
                        p_sb = ppool.tile([128, 1024], BF16, tag="p")
                        nc.scalar.activation(
                            p_sb, s_ps, AF.Exp,
                            bias=mbias_sb[:, kt:kt + 1], scale=SCALE,
                        )
                        # O_T (parts 64:128) + replicated rowsum (parts 0:64)
                        for i in range(2):
                            nc.tensor.matmul(
                                o_ps[i],
                                lhsT=v_aug[:, kt, 2 * hp + i, :],
                                rhs=p_sb[:, i * 512:(i + 1) * 512],
                                start=(kt == 0), stop=(kt == kt_c - 1),
                            )
                        if kt % 2 == 1 and pending:
                            emit_piece()
                    # O_final = 0.9*O_T/rs + 0.1*vmean
                    for i in range(2):
                        rr = divpool.tile([128, 512], F32, tag="rr")
                        nc.vector.reciprocal_approx_fast(
                            rr[0:64, :], o_ps[i][0:64, :])
                        t1 = divpool.tile([128, 512], F32, tag="t1")
                        nc.vector.tensor_mul(
                            t1[0:64, :], o_ps[i][64:128, :], rr[0:64, :])
                        nc.gpsimd.tensor_scalar(
                            ofin[hp][64 * i:64 * (i + 1), qsl], t1[0:64, :],
                            1.0 - EPS_SMOOTH,
                            vmean_sb[hp][64 * i:64 * (i + 1), :],
                            ALU.mult, ALU.add,
                        )
                # interleave: q projection for the next chunk; this chunk's
                # out-projection pieces drain inside the next chunk's kt loop
                if qc + 1 < QC:
                    proj_block("q", xq_sb, qTn, (qc + 1) * 512, 512)
                queue_out_proj(qc)
                while len(pending) > 8:
                    emit_piece()
            # flush the last chunk's pieces, double-buffered across two tags
            j = 0
            while pending:
                emit_piece(*(("op", 1) if j % 2 == 0 else ("o", 3)))
                j += 1

    nc.finalize()
    return nc


_NC_CACHE = {}


def _get_nc(kt_c):
    if kt_c not in _NC_CACHE:
        _NC_CACHE[kt_c] = _build_nc(kt_c)
    return _NC_CACHE[kt_c]


def kernel(q_in, k_in, v_in, kv_pad_mask, Wq, bq, Wk, bk, Wv, bv, Wo, bo,
           _trace=False):
    f32 = np.float32
    bf = ml_dtypes.bfloat16
    q_in = np.asarray(q_in, f32)
    k_in = np.asarray(k_in, f32)
    v_in = np.asarray(v_in, f32)
    mask = np.asarray(kv_pad_mask, bool)
    Wq, bq, Wk, bk, Wv, bv, Wo, bo = (
        np.asarray(a, f32) for a in (Wq, bq, Wk, bk, Wv, bv, Wo, bo)
    )

    # key compaction: gather valid keys per batch, pad to a tile multiple
    idx = [np.flatnonzero(~mask[b]) for b in range(B)]
    nv = [len(ix) for ix in idx]
    kt_c = max(1, max((n + 127) // 128 for n in nv))
    LKC = kt_c * 128
    nc = _get_nc(kt_c)

    xT = {}
    mb = {}
    for b in range(B):
        kc = np.zeros((LKC, D), f32)
        kc[:nv[b]] = k_in[b][idx[b]]
        vc = np.zeros((LKC, D), f32)
        vc[:nv[b]] = v_in[b][idx[b]]
        xT[("q", b)] = np.ascontiguousarray(q_in[b].T).astype(bf)
        xT[("k", b)] = np.ascontiguousarray(kc.T).astype(bf)
        xT[("v", b)] = np.ascontiguousarray(vc.T).astype(bf)
        mvalid = np.zeros(LKC, f32)
        mvalid[nv[b]:] = MASK_BIAS
        mb[b] = np.ascontiguousarray(mvalid.reshape(kt_c, 128).T)

    in_maps = []
    for core in range(N_CORES):
        b = core // 4
        h0 = (core % 4) * HEADS_PER_CORE
        rows = slice(h0 * DH, h0 * DH + HPC)
        valid = (~mask[b]).astype(f32)
        nvb = max(float(valid.sum()), 1.0)
        vscaled = valid * (EPS_SMOOTH / nvb)
        # 0.1 * mean_over_valid(v) for this core's 256 dims
        vm = (vscaled @ v_in[b]) @ Wv[rows].T + EPS_SMOOTH * bv[rows]
        in_maps.append({
            "xqT": xT[("q", b)],
            "xkT": xT[("k", b)],
            "xvT": xT[("v", b)],
            "wq_t": np.ascontiguousarray(Wq[rows].T).astype(bf),
            "wk_t": np.ascontiguousarray(Wk[rows].T).astype(bf),
            "wv_t": np.ascontiguousarray(Wv[rows].T).astype(bf),
            "wo_t": np.ascontiguousarray(Wo[:, rows].T).astype(bf),
            "bq": np.ascontiguousarray(bq[rows].reshape(2, 1, 128)).astype(bf),
            "bk": np.ascontiguousarray(bk[rows].reshape(2, 1, 128)).astype(bf),
            "bv": np.ascontiguousarray(bv[rows].reshape(1, HPC)).astype(bf),
            "mbias": mb[b],
            "vmean": np.ascontiguousarray(vm.astype(f32).reshape(2, 128, 1)),
        })

    res = run_bass_kernel_spmd(nc, in_maps, core_ids=list(range(N_CORES)),
                               trace=_trace)
    out = np.zeros((B, L, D), f32)
    for core in range(N_CORES):
        out[core // 4] += res.results[core]["partial"]
    out += bo[None, None, :]
    if _trace:
        kernel._last_result = res
    return out
